# revision 15
# baseline (speedup 1.0000x reference)
"""AxialSelfAttention2d Trainium kernel (8-core SPMD, single launch).

Strategy:
  Phase 1 (row attention over L): shard E=128 -> 16 rows/core.
  AllToAll reshard -> Phase 2 (col attention over E): shard L=256 -> 32 cols/core.

Device layout tricks:
  - Host pre-transposes x and the QKV weights, so phase-1 matmuls need no
    on-device transpose at all.
  - q/k are produced transposed ([d_out, token]); v is produced natural
    ([token, d_out]) with a ones-column appended so the softmax denominator
    falls out of the attention matmul for free.
  - Scores are computed as S^T (keys on partitions) so the padding mask and
    the 1/sqrt(dh) scale fold into the single Exp activation (bias+scale).
  - Matmuls run in float32r (full PE rate at N>=256, fp32-ish precision).
"""

import sys

sys.path.insert(0, "/opt/trn_rl_repo")

import numpy as np

import concourse.bass as bass
from concourse import bacc
import concourse.tile as tile
from concourse import mybir
from concourse.bass_utils import run_bass_kernel_spmd

H, DH = 12, 64
D = H * DH           # 768
E, L = 128, 256
NC = 8
E_SH = E // NC       # 16 rows per core, phase 1
L_SH = L // NC       # 32 cols per core, phase 2
LG = 2               # phase-2 column group (batches matmul N to 256)
NEG = -10000.0
EPS = 1e-5
SCALE = DH ** -0.5
KO = D // 128        # 6 contraction subtiles

f32 = mybir.dt.float32
f32r = mybir.dt.float32r
FT = mybir.ActivationFunctionType


def _bcast_dram(handle, n_part, free):
    """DMA-source AP replicating a [free] DRAM vector across n_part partitions."""
    ap = handle.ap()
    return bass.AP(tensor=ap.tensor, offset=ap.offset, ap=[[0, n_part], [1, free]])


def build_kernel(use_br, use_bc, use_g1, use_g2):
    nc = bacc.Bacc("TRN2", target_bir_lowering=False, debug=False, num_devices=8)

    xT = nc.dram_tensor("xT", [E_SH, D, L], f32r, kind="ExternalInput")
    xn = nc.dram_tensor("xn", [E_SH, L, D], f32, kind="ExternalInput")
    wrT = nc.dram_tensor("wrT", [D, 3 * D], f32r, kind="ExternalInput")
    wcT = nc.dram_tensor("wcT", [D, 3 * D], f32r, kind="ExternalInput")
    negr = nc.dram_tensor("negr", [E_SH, 128, 2], f32, kind="ExternalInput")
    keepc = nc.dram_tensor("keepc", [E, L_SH], f32, kind="ExternalInput")
    brow = nc.dram_tensor("brow", [3 * D], f32, kind="ExternalInput")
    bcol = nc.dram_tensor("bcol", [3 * D], f32, kind="ExternalInput")
    g1 = nc.dram_tensor("g1", [D], f32, kind="ExternalInput")
    be1 = nc.dram_tensor("be1", [D], f32, kind="ExternalInput")
    g2 = nc.dram_tensor("g2", [D], f32, kind="ExternalInput")
    be2 = nc.dram_tensor("be2", [D], f32, kind="ExternalInput")
    identd = nc.dram_tensor("identd", [128, 128], f32, kind="ExternalInput")
    out = nc.dram_tensor("out", [E, L_SH, D], f32, kind="ExternalOutput")

    with tile.TileContext(nc) as tc:
        with (
            tc.tile_pool(name="wp", bufs=1) as wp,
            tc.tile_pool(name="const", bufs=1) as const,
            tc.tile_pool(name="sb", bufs=2) as sb,
            tc.tile_pool(name="ptp", bufs=3) as ptp,
            tc.tile_pool(name="small", bufs=3) as small,
            tc.tile_pool(name="ps", bufs=2, space="PSUM") as ps,
            tc.tile_pool(name="dram", bufs=1, space="DRAM") as dram,
        ):
            # ---------------- persistent state ----------------
            w_sb = wp.tile([128, KO, 3 * D], f32r, tag="w", name="wrow")
            nc.sync.dma_start(
                out=w_sb[:], in_=wrT.ap().rearrange("(ko p) m -> p ko m", p=128)
            )
            ident = const.tile([128, 128], f32)
            nc.sync.dma_start(out=ident[:], in_=identd[:, :])
            eps_sb = const.tile([128, 1], f32)
            nc.vector.memset(eps_sb, EPS)
            keep_sb = const.tile([E, L_SH], f32)
            nc.sync.dma_start(out=keep_sb[:], in_=keepc[:, :])

            def ln_vec(handle):
                t = const.tile([128, D], f32, name=handle.name + "_bc")
                nc.sync.dma_start(out=t[:], in_=_bcast_dram(handle, 128, D))
                return t

            g1_sb = ln_vec(g1) if use_g1 else None
            be1_sb = ln_vec(be1) if use_g1 else None
            g2_sb = ln_vec(g2) if use_g2 else None
            be2_sb = ln_vec(be2) if use_g2 else None

            def qkbias(handle):
                # [128, 12] per-partition bias for the transposed q/k tiles
                t = const.tile([128, 12], f32, name=handle.name + "_qk")
                nc.sync.dma_start(
                    out=t[:],
                    in_=handle.ap()[: 2 * D].rearrange("(dt p) -> p dt", p=128),
                )
                return t

            def vbias(handle):
                t = const.tile([128, D], f32, name=handle.name + "_v")
                ap = handle.ap()
                vap = bass.AP(
                    tensor=ap.tensor, offset=2 * D, ap=[[0, 128], [1, D]]
                )
                nc.sync.dma_start(out=t[:], in_=vap)
                return t

            br_qk = qkbias(brow) if use_br else None
            br_v = vbias(brow) if use_br else None
            bc_qk = qkbias(bcol) if use_bc else None
            bc_v = vbias(bcol) if use_bc else None

            a2a_in = dram.tile([NC, E_SH, L_SH, D], f32)
            a2a_out = dram.tile([NC, E_SH, L_SH, D], f32)

            def copyback(dst, src, bias):
                if bias is None:
                    nc.any.tensor_copy(out=dst, in_=src)
                else:
                    nc.vector.tensor_scalar_add(out=dst, in0=src, scalar1=bias)

            def layernorm(res, g_sb, b_sb):
                # res: [128, D] sbuf, normalized in place over free dim
                stats = small.tile([128, 3, nc.vector.BN_STATS_DIM], f32, tag="bnst")
                for i in range(3):
                    nc.vector.bn_stats(
                        out=stats[:, i, :], in_=res[:, i * 256:(i + 1) * 256]
                    )
                mv = small.tile([128, nc.vector.BN_AGGR_DIM], f32, tag="bnmv")
                nc.vector.bn_aggr(out=mv[:], in_=stats[:])
                nc.scalar.activation(
                    out=mv[:, 1:2], in_=mv[:, 1:2], func=FT.Sqrt, bias=eps_sb[:],
                )
                nc.vector.reciprocal(out=mv[:, 1:2], in_=mv[:, 1:2])
                nc.vector.tensor_scalar(
                    out=res, in0=res, scalar1=mv[:, 0:1], scalar2=mv[:, 1:2],
                    op0=mybir.AluOpType.subtract, op1=mybir.AluOpType.mult,
                )
                if g_sb is not None:
                    nc.vector.tensor_mul(out=res, in0=res, in1=g_sb[:])
                    nc.vector.tensor_add(out=res, in0=res, in1=b_sb[:])

            def attn_epilogue(avs, res_slice):
                # avs: 3 psum tiles [128, 512], 4 head-slots of 128 each
                av_sb = small.tile([128, H, 128], f32, tag="avsb", bufs=2)
                for t in range(3):
                    nc.any.tensor_copy(
                        out=av_sb[:, 4 * t:4 * (t + 1), :],
                        in_=avs[t].rearrange("p (h c) -> p h c", c=128),
                    )
                rz = small.tile([128, H], f32, tag="rz")
                nc.vector.reciprocal(out=rz[:], in_=av_sb[:, :, 64])
                nc.vector.tensor_tensor(
                    res_slice.rearrange("p (h c) -> p h c", c=DH),
                    av_sb[:, :, 0:DH],
                    rz[:, :, None].to_broadcast([128, H, DH]),
                    mybir.AluOpType.mult,
                )

            # ---------------- phase 1: row attention ----------------
            for e in range(E_SH):
                xT_e = sb.tile([128, KO, L], f32r, tag="xT")
                nc.sync.dma_start(
                    out=xT_e[:], in_=xT[e].rearrange("(ko p) t -> p ko t", p=128)
                )
                negr_e = small.tile([128, 2], f32, tag="negr")
                nc.sync.dma_start(out=negr_e[:], in_=negr[e])

                # q/k transposed: [do, t] for do < 1536
                qk_sb = sb.tile([128, 12, L], f32r, tag="qk")
                for dt in range(12):
                    qk_ps = ps.tile([128, L], f32, tag="mm")
                    for ko in range(KO):
                        nc.tensor.matmul(
                            qk_ps[:],
                            w_sb[:, ko, dt * 128:(dt + 1) * 128],
                            xT_e[:, ko],
                            start=(ko == 0), stop=(ko == KO - 1),
                        )
                    copyback(
                        qk_sb[:, dt], qk_ps[:],
                        br_qk[:, dt:dt + 1] if use_br else None,
                    )

                # v natural: [t, dv] + ones column per head
                v_sb = sb.tile([128, 2, H, 65], f32r, tag="v")
                nc.vector.memset(v_sb[:], 1.0)
                for jt in range(2):
                    for c0, cw in ((0, 512), (512, 256)):
                        v_ps = ps.tile([128, 512], f32, tag="mm", name="v_ps")[:, :cw]
                        for ko in range(KO):
                            nc.tensor.matmul(
                                v_ps,
                                xT_e[:, ko, jt * 128:(jt + 1) * 128],
                                w_sb[:, ko, 2 * D + c0:2 * D + c0 + cw],
                                start=(ko == 0), stop=(ko == KO - 1),
                            )
                        nc.any.tensor_copy(
                            out=v_sb[:, jt, c0 // 64:(c0 + cw) // 64, 0:64],
                            in_=v_ps.rearrange("p (h c) -> p h c", c=64),
                        )
                    if use_br:
                        nc.vector.tensor_add(
                            out=v_sb[:, jt, :, 0:64],
                            in0=v_sb[:, jt, :, 0:64],
                            in1=br_v[:].rearrange("p (h c) -> p h c", c=64),
                        )

                # scores S^T = K^T-tiles x Q^T, exp with mask-bias, per jt
                q64 = sb.tile([64, 12, L], bf16, tag="q64")
                k64 = sb.tile([64, 12, L], bf16, tag="k64")
                nc.sync.dma_start(out=q64[:, 0:12:2, :], in_=qk_sb[0:64, 0:6, :])
                nc.sync.dma_start(out=q64[:, 1:12:2, :], in_=qk_sb[64:128, 0:6, :])
                nc.sync.dma_start(out=k64[:, 0:12:2, :], in_=qk_sb[0:64, 6:12, :])
                nc.sync.dma_start(out=k64[:, 1:12:2, :], in_=qk_sb[64:128, 6:12, :])
                pt = [None, None]
                for jt in range(2):
                    pt[jt] = ptp.tile([128, H, L], f32r, tag="pt", name="pt")
                    for m in range(6):  # head pairs (2m, 2m+1) share dt
                        st_ps = ps.tile([128, 512], f32, tag="st")
                        for hi in range(2):
                            h = 2 * m + hi
                            nc.tensor.matmul(
                                st_ps[:, hi * 256:(hi + 1) * 256],
                                k64[:, h, jt * 128:(jt + 1) * 128],
                                q64[:, h, :],
                                start=True, stop=True,
                            )
                        nc.scalar.activation(
                            out=pt[jt][:, 2 * m:2 * m + 2, :], in_=st_ps[:],
                            func=FT.Exp, bias=negr_e[:, jt:jt + 1], scale=SCALE,
                        )

                res_sb = sb.tile([128, 2, D], f32, tag="res")
                x_e = sb.tile([128, 2, D], f32, tag="xe")
                nc.sync.dma_start(
                    out=x_e[:], in_=xn[e].rearrange("(it p) d -> p it d", p=128)
                )
                for it in range(2):
                    avs = [
                        ps.tile([128, 512], f32, tag="av", bufs=3, name="av")
                        for _ in range(3)
                    ]
                    for h in range(H):
                        dst = avs[h // 4][:, (h % 4) * 128:(h % 4) * 128 + 65]
                        for jt in range(2):
                            nc.tensor.matmul(
                                dst,
                                pt[jt][:, h, it * 128:(it + 1) * 128],
                                v_sb[:, jt, h, 0:65],
                                start=(jt == 0), stop=(jt == 1),
                            )
                    attn_epilogue(avs, res_sb[:, it])
                    nc.vector.tensor_add(
                        out=res_sb[:, it], in0=res_sb[:, it], in1=x_e[:, it]
                    )
                    layernorm(res_sb[:, it], g1_sb, be1_sb)
                    # scatter into alltoall send buffer: [dest, e, l_local, d]
                    nc.sync.dma_start(
                        out=a2a_in[it * 4:(it + 1) * 4, e],
                        in_=res_sb[:, it],
                    )

            # ---------------- reshard ----------------
            wc_sb = wp.tile([128, KO, 3 * D], f32r, tag="w", name="wcol")
            nc.sync.dma_start(
                out=wc_sb[:], in_=wcT.ap().rearrange("(ko p) m -> p ko m", p=128)
            )
            nc.gpsimd.collective_compute(
                "AllToAll", mybir.AluOpType.bypass,
                replica_groups=[list(range(NC))],
                ins=[a2a_in[:].opt()], outs=[a2a_out[:].opt()],
            )

            # ---------------- phase 2: column attention ----------------
            o1_view = a2a_out[:].rearrange("s ee l d -> (s ee) l d")
            for lg in range(L_SH // LG):
                o1_sb = sb.tile([128, LG, D], f32, tag="xe")
                for li in range(LG):
                    nc.sync.dma_start(
                        out=o1_sb[:, li], in_=o1_view[:, lg * LG + li]
                    )
                # transpose tokens: o1T [d-part, ko, t] (t = li*128 + e)
                o1T = sb.tile([128, KO, LG * 128], f32r, tag="xT")
                for li in range(LG):
                    for kp in range(KO // 2):
                        t_ps = ps.tile([128, 256], f32, tag="mm")
                        for k2 in range(2):
                            nc.tensor.transpose(
                                t_ps[:, k2 * 128:(k2 + 1) * 128],
                                o1_sb[:, li, (2 * kp + k2) * 128:
                                      (2 * kp + k2 + 1) * 128],
                                ident[:],
                            )
                        nc.any.tensor_copy(
                            out=o1T[:, 2 * kp:2 * kp + 2,
                                    li * 128:(li + 1) * 128],
                            in_=t_ps.rearrange("p (k t) -> p k t", t=128),
                        )

                qkc = sb.tile([128, 12, LG * 128], f32r, tag="qk")
                for dt in range(12):
                    qk_ps = ps.tile([128, LG * 128], f32, tag="mm")
                    for ko in range(KO):
                        nc.tensor.matmul(
                            qk_ps[:],
                            wc_sb[:, ko, dt * 128:(dt + 1) * 128],
                            o1T[:, ko],
                            start=(ko == 0), stop=(ko == KO - 1),
                        )
                    copyback(
                        qkc[:, dt], qk_ps[:],
                        bc_qk[:, dt:dt + 1] if use_bc else None,
                    )

                v2 = sb.tile([128, LG, H, 65], f32r, tag="v")
                nc.vector.memset(v2[:], 1.0)
                for li in range(LG):
                    for c0, cw in ((0, 512), (512, 256)):
                        v_ps = ps.tile([128, 512], f32, tag="mm", name="v_ps")[:, :cw]
                        for ko in range(KO):
                            nc.tensor.matmul(
                                v_ps,
                                o1T[:, ko, li * 128:(li + 1) * 128],
                                wc_sb[:, ko, 2 * D + c0:2 * D + c0 + cw],
                                start=(ko == 0), stop=(ko == KO - 1),
                            )
                        nc.any.tensor_copy(
                            out=v2[:, li, c0 // 64:(c0 + cw) // 64, 0:64],
                            in_=v_ps.rearrange("p (h c) -> p h c", c=64),
                        )
                    if use_bc:
                        nc.vector.tensor_add(
                            out=v2[:, li, :, 0:64],
                            in0=v2[:, li, :, 0:64],
                            in1=bc_v[:].rearrange("p (h c) -> p h c", c=64),
                        )
                    # padding mask: zero out masked key rows (incl. ones col)
                    nc.vector.tensor_scalar_mul(
                        out=v2[:, li], in0=v2[:, li],
                        scalar1=keep_sb[:, lg * LG + li:lg * LG + li + 1],
                    )

                qc64 = sb.tile([64, 12, LG * 128], bf16, tag="q64", name="qc64")
                kc64 = sb.tile([64, 12, LG * 128], bf16, tag="k64", name="kc64")
                nc.sync.dma_start(out=qc64[:, 0:12:2, :], in_=qkc[0:64, 0:6, :])
                nc.sync.dma_start(out=qc64[:, 1:12:2, :], in_=qkc[64:128, 0:6, :])
                nc.sync.dma_start(out=kc64[:, 0:12:2, :], in_=qkc[0:64, 6:12, :])
                nc.sync.dma_start(out=kc64[:, 1:12:2, :], in_=qkc[64:128, 6:12, :])
                res2 = sb.tile([128, LG, D], f32, tag="res")
                for li in range(LG):
                    pt2 = [None] * H
                    for m in range(6):
                        st_ps = ps.tile([128, 256], f32, tag="st")
                        for hi in range(2):
                            h = 2 * m + hi
                            nc.tensor.matmul(
                                st_ps[:, hi * 128:(hi + 1) * 128],
                                kc64[:, h, li * 128:(li + 1) * 128],
                                qc64[:, h, li * 128:(li + 1) * 128],
                                start=True, stop=True,
                            )
                        ptt = ptp.tile([128, 2, 128], f32r, tag="pt2")
                        nc.scalar.activation(
                            out=ptt[:], in_=st_ps[:], func=FT.Exp, scale=SCALE,
                        )
                        pt2[2 * m] = ptt[:, 0]
                        pt2[2 * m + 1] = ptt[:, 1]

                    avs = [
                        ps.tile([128, 512], f32, tag="av", bufs=3, name="av")
                        for _ in range(3)
                    ]
                    for h in range(H):
                        dst = avs[h // 4][:, (h % 4) * 128:(h % 4) * 128 + 65]
                        nc.tensor.matmul(
                            dst, pt2[h], v2[:, li, h, 0:65],
                            start=True, stop=True,
                        )
                    attn_epilogue(avs, res2[:, li])
                    nc.vector.tensor_add(
                        out=res2[:, li], in0=res2[:, li], in1=o1_sb[:, li]
                    )
                    layernorm(res2[:, li], g2_sb, be2_sb)
                    nc.sync.dma_start(
                        out=out[:, lg * LG + li, :], in_=res2[:, li]
                    )

    nc.finalize()
    return nc


import jax
from jax.sharding import Mesh, PartitionSpec
from jax.experimental.shard_map import shard_map
from concourse import bass2jax


def _make_runner(nc):
    """Mirror bass2jax.run_bass_via_pjrt, but keep the jitted callable so
    repeat kernel() calls don't recompile."""
    bass2jax.install_neuronx_cc_hook()
    partition_name = (
        nc.partition_id_tensor.name if nc.partition_id_tensor else None
    )
    in_names, out_names, out_avals = [], [], []
    for alloc in nc.m.functions[0].allocations:
        if not isinstance(alloc, mybir.MemoryLocationSet):
            continue
        name = alloc.memorylocations[0].name
        if alloc.kind == "ExternalInput":
            if name != partition_name:
                in_names.append(name)
        elif alloc.kind == "ExternalOutput":
            out_names.append(name)
            out_avals.append(
                jax.core.ShapedArray(
                    tuple(alloc.tensor_shape), mybir.dt.np(alloc.dtype)
                )
            )
    n_params = len(in_names)
    n_outs = len(out_avals)
    all_names = list(in_names) + list(out_names)
    if partition_name is not None:
        all_names.append(partition_name)
    donate = tuple(range(n_params, n_params + n_outs))

    def _body(*args):
        operands = list(args)
        if partition_name is not None:
            operands.append(bass2jax.partition_id_tensor())
        outs = bass2jax._bass_exec_p.bind(
            *operands,
            out_avals=tuple(out_avals),
            in_names=tuple(all_names),
            out_names=tuple(out_names),
            lowering_input_output_aliases=(),
            sim_require_finite=True,
            sim_require_nnan=True,
            nc=nc,
        )
        return tuple(outs)

    mesh = Mesh(np.asarray(jax.devices()[:NC]), ("core",))
    in_specs = (PartitionSpec("core"),) * (n_params + n_outs)
    out_specs = (PartitionSpec("core"),) * n_outs
    sharded = jax.jit(
        shard_map(
            _body, mesh=mesh, in_specs=in_specs, out_specs=out_specs,
            check_rep=False,
        ),
        donate_argnums=donate,
        keep_unused=True,
    )
    return sharded, in_names, out_names, out_avals, mesh


_CACHE = {}
TRACE = False
LAST = {}


def kernel(x, w_row, b_row, w_col, b_col, g1, beta1, g2, beta2, padding_mask):
    x = np.asarray(x, dtype=np.float32)
    w_row = np.asarray(w_row, dtype=np.float32)
    w_col = np.asarray(w_col, dtype=np.float32)
    b_row = np.asarray(b_row, dtype=np.float32)
    b_col = np.asarray(b_col, dtype=np.float32)
    g1 = np.asarray(g1, dtype=np.float32)
    beta1 = np.asarray(beta1, dtype=np.float32)
    g2 = np.asarray(g2, dtype=np.float32)
    beta2 = np.asarray(beta2, dtype=np.float32)
    mask = np.asarray(padding_mask)

    use_br = not np.all(b_row == 0.0)
    use_bc = not np.all(b_col == 0.0)
    use_g1 = not (np.all(g1 == 1.0) and np.all(beta1 == 0.0))
    use_g2 = not (np.all(g2 == 1.0) and np.all(beta2 == 0.0))

    key = (use_br, use_bc, use_g1, use_g2)
    if key not in _CACHE:
        _CACHE[key] = _make_runner(build_kernel(*key))
    runner = _CACHE[key]

    neg = np.where(mask[0], np.float32(NEG), np.float32(0.0)).astype(np.float32)
    keep = np.where(mask[0], np.float32(0.0), np.float32(1.0)).astype(np.float32)
    wrT = np.ascontiguousarray(w_row.T)
    wcT = np.ascontiguousarray(w_col.T)

    in_maps = []
    for c in range(NC):
        rows = slice(E_SH * c, E_SH * (c + 1))
        cols = slice(L_SH * c, L_SH * (c + 1))
        in_maps.append({
            "xT": np.ascontiguousarray(x[0, rows].transpose(0, 2, 1)),
            "xn": np.ascontiguousarray(x[0, rows]),
            "wrT": wrT,
            "wcT": wcT,
            "negr": np.ascontiguousarray(
                neg[rows].reshape(E_SH, 2, 128).transpose(0, 2, 1)
            ),
            "keepc": np.ascontiguousarray(keep[:, cols]),
            "brow": b_row, "bcol": b_col,
            "g1": g1, "be1": beta1, "g2": g2, "be2": beta2,
            "identd": np.eye(128, dtype=np.float32),
        })

    sharded, in_names, out_names, out_avals, mesh = runner
    concat_in = [
        np.concatenate([m[name] for m in in_maps], axis=0) for name in in_names
    ]
    concat_zeros = [
        np.zeros((NC * a.shape[0], *a.shape[1:]), a.dtype) for a in out_avals
    ]
    out_arrs = sharded(*concat_in, *concat_zeros)
    LAST["runner"] = runner
    LAST["concat_in"] = concat_in
    LAST["out_shapes"] = [
        (NC * a.shape[0], *a.shape[1:]) for a in out_avals
    ]
    oi = out_names.index("out")
    res = np.asarray(out_arrs[oi]).reshape(NC, E, L_SH, D)
    full = np.empty((1, E, L, D), dtype=np.float32)
    for c in range(NC):
        full[0, :, L_SH * c:L_SH * (c + 1), :] = res[c]
    return full


def bench(n=3):
    """Re-run the compiled kernel with device-resident inputs; returns
    per-call wall seconds (dispatch + device execution, no H2D of inputs)."""
    import time as _time
    sharded, in_names, out_names, out_avals, mesh = LAST["runner"]
    from jax.sharding import NamedSharding
    spec = NamedSharding(mesh, PartitionSpec("core"))
    dev_in = [jax.device_put(a, spec) for a in LAST["concat_in"]]
    jax.block_until_ready(dev_in)
    times = []
    for _ in range(n):
        dz = [
            jax.device_put(np.zeros(s, a.dtype), spec)
            for s, a in zip(LAST["out_shapes"], out_avals)
        ]
        jax.block_until_ready(dz)
        t0 = _time.perf_counter()
        out = sharded(*dev_in, *dz)
        jax.block_until_ready(out)
        times.append(_time.perf_counter() - t0)
    return times
